# revision 47
# baseline (speedup 1.0000x reference)
"""TRN2 Bass kernel for nn_DecoderLayer_70781061038465 (Falcon-7B style decoder
layer: fractured LayerNorm -> parallel MQA attention + MLP -> residual).

Sharding: 8-way tensor parallelism, no collectives. Each core computes a
partial sum of (attn_out + mlp_out) over its head/MLP shard; the host reduces
the 8 partials and adds the residual.

Per-core math (all LN work folded into matmuls):
  - LN affine folded into projection weights (columns scaled by ln_w; ln_b
    enters via a bias row consumed by an all-ones contraction row).
  - mean/rstd correction folded via (a) pre-scaling token rows by rstd and
    (b) a -mu*rstd contraction row whose weight-row is the column-sum of the
    ln_w-scaled weights.
  - softmax 1/sqrt(64) folded into wq.

Attention runs fully transposed: scoresT[sk,sq] come straight off the PE,
exp is applied without max-subtraction (score range is bounded for this
problem), the softmax denominator rides along as an all-ones 65th column of
V, and normalization happens on the token-major context eviction.

Structure tuned for the TimelineSim cost model:
  - x-tilde and context transposes are single batched XBAR instructions.
  - weights live in DRAM pre-tiled so every weight DMA is one descriptor
    run per partition (contiguous >= 1KB).
  - mlp/dense weight stream issues from the (otherwise idle) GPSIMD queue.
  - attention head loop is software-pipelined (scores/exp of head h issue
    before ctx of head h-1) so the PE never waits on the Act-engine exp.
  - dense contraction runs gelu tiles first, ctx tiles last.
  - output stores are bf16, one DMA per 512-column stripe.
"""
import sys
if "/opt/trn_rl_repo" not in sys.path:
    sys.path.insert(0, "/opt/trn_rl_repo")

from contextlib import ExitStack

import numpy as np
import ml_dtypes

import concourse.bass as bass
import concourse.tile as tile
from concourse import bacc, mybir
from concourse.bass_utils import run_bass_kernel_spmd

F32 = mybir.dt.float32
BF16 = mybir.dt.bfloat16
AX = mybir.AxisListType.X
AF = mybir.ActivationFunctionType
MUL = mybir.AluOpType.mult
ADD = mybir.AluOpType.add

# problem shapes (hardcoded per contract)
B, S, H, NH, HD = 2, 1024, 4544, 71, 64
T = B * S                 # 2048 tokens
HP = 4608                 # padded hidden (36*128)
KT = HP // 128            # 36 contraction tiles
NHP = 80                  # padded heads total
NHC = 10                  # heads per core
QC = NHC * HD             # 640 q channels/core
F4 = 4 * H                # 18176
F4C_REAL = F4 // 8        # 2272
F4C = 2304                # padded (18*128)
OC = QC + 128 + F4C       # 3072 proj out channels (q | k,v | h4h)
MT = OC // 128            # 24 proj m-tiles
DK = (QC + F4C) // 128    # 23 dense+down contraction tiles (18 gelu + 5 ctx)
FC = HP // 512            # 9 output f-chunks
KCH = [(k, min(k + 2, 23)) for k in range(0, 23, 2)]
EPS = 1e-5
NEG = -30.0

_CACHE = {}


def _et_idx(skt, sqc):
    # compact causal score-tile layout: sqc=0 holds skt 0..3, sqc=1 all 8
    return skt if sqc == 0 else 4 + skt


def _build():
    nc = bacc.Bacc("TRN2", target_bir_lowering=False, debug=False)
    xb_d = nc.dram_tensor("xb", [T, HP], BF16, kind="ExternalInput")
    wpk_d = nc.dram_tensor("wpk", [MT, 128, KT * 128], BF16,
                           kind="ExternalInput")
    wdd_d = nc.dram_tensor("wdd", [FC, DK, 128, 512], BF16,
                           kind="ExternalInput")
    cs_d = nc.dram_tensor("csn", [2, 64, S], BF16, kind="ExternalInput")
    dm_d = nc.dram_tensor("dmask", [128, 128], BF16, kind="ExternalInput")
    out_d = nc.dram_tensor("out", [T, HP], BF16, kind="ExternalOutput")

    xb = xb_d.ap()
    out = out_d.ap()

    with tile.TileContext(nc) as tc, ExitStack() as ctx:
        def pool(name, bufs, space="SBUF"):
            return ctx.enter_context(tc.tile_pool(name=name, bufs=bufs, space=space))

        const = pool("const", 1)
        xin = pool("xin", 4)
        xtp = pool("xtp", 1)
        wpool = pool("wp", 3)
        res = pool("res", 1)      # per-batch residents: qt/kt/vt/gt/ct
        et_p = pool("et", 2)
        c2_p = pool("c2", 1)
        rotp = pool("rot", 1)
        wdp = pool("wdp", 2)
        outp = pool("outp", 1)
        small = pool("small", 4)
        psp = pool("psp", 8, space="PSUM")

        cos_sb_t = const.tile([64, S], BF16, tag="cos")
        nc.sync.dma_start(cos_sb_t[:], cs_d.ap()[0])
        sin_sb_t = const.tile([64, S], BF16, tag="sin")
        nc.sync.dma_start(sin_sb_t[:], cs_d.ap()[1])
        cos_sb = cos_sb_t[:]
        sin_sb = sin_sb_t[:]
        dmaskT = const.tile([128, 128], BF16, tag="dmaskT")
        nc.sync.dma_start(dmaskT[:], dm_d.ap())

        for b in range(B):
            qt = res.tile([64, NHC, S], BF16, tag="qt")
            kt = res.tile([64, S], BF16, tag="kt")
            vt = res.tile([128, 8, 72], BF16, tag="vt")
            gt = res.tile([128, 18, S], BF16, tag="gt")
            ct = res.tile([128, 5, S], BF16, tag="ct")
            nc.vector.memset(vt[:, :, 64:65], 1.0)   # denominator ones-column

            # ---- Phase A: LN stats + rstd-scale, half-row pipeline, one
            # batched transpose per half-row ----
            HH = HP // 2              # 2304 = 18 k-tiles
            HB2 = H - HH              # 2240 real cols in second half
            xt = xtp.tile([128, KT, S], BF16, tag="xt")
            for r in range(8):
                row0 = b * S + r * 128
                xra = xin.tile([128, HH], BF16, tag="xrow")
                nc.sync.dma_start(xra[:], xb[row0:row0 + 128, :HH])
                xrb = xin.tile([128, HH], BF16, tag="xrow")
                nc.sync.dma_start(xrb[:], xb[row0:row0 + 128, HH:])
                st = small.tile([128, 16, 6], F32, tag="st")
                xga = xra.rearrange("p (g d) -> p g d", g=8)
                xgb = xrb[:, :HB2].rearrange("p (g d) -> p g d", g=8)
                for g in range(8):
                    nc.vector.bn_stats(st[:, g, :], xga[:, g, :])
                    nc.vector.bn_stats(st[:, 8 + g, :], xgb[:, g, :])
                mv = small.tile([128, 2], F32, tag="mv")
                nc.vector.bn_aggr(mv[:], st[:])
                rstd = small.tile([128, 1], F32, tag="rstd")
                nc.vector.tensor_scalar_add(rstd[:], mv[:, 1:2], EPS)
                nc.scalar.activation(rstd[:], rstd[:], AF.Sqrt)
                nc.vector.reciprocal(rstd[:], rstd[:])
                mr = small.tile([128, 1], F32, tag="mr")
                nc.vector.tensor_tensor(mr[:], mv[:, 0:1], rstd[:], op=MUL)
                nc.vector.tensor_scalar_mul(mr[:], mr[:], -1.0)
                nc.vector.tensor_scalar_mul(xra[:], xra[:], rstd[:])
                nc.scalar.dma_start(xt[:, :18, r * 128:(r + 1) * 128], xra[:],
                                    transpose=True)
                nc.vector.tensor_scalar_mul(xrb[:, :HB2], xrb[:, :HB2],
                                            rstd[:])
                nc.vector.memset(xrb[:, HB2:HB2 + 1], 1.0)
                nc.vector.tensor_copy(xrb[:, HB2 + 1:HB2 + 2], mr[:])
                nc.scalar.dma_start(xt[:, 18:, r * 128:(r + 1) * 128], xrb[:],
                                    transpose=True)

            # ---- Phase B: projections (feature-major q/k/g, token-major v) --
            for m in range(MT):
                wta = wpool.tile([128, 18, 128], BF16, tag="wt")
                nc.sync.dma_start(
                    wta[:], wpk_d.ap()[m][:, :18 * 128].rearrange(
                        "p (k c) -> p k c", c=128))
                wtb = wpool.tile([128, 18, 128], BF16, tag="wt")
                nc.sync.dma_start(
                    wtb[:], wpk_d.ap()[m][:, 18 * 128:].rearrange(
                        "p (k c) -> p k c", c=128))

                for hb in range(2):
                    hcols = slice(hb * 512, hb * 512 + 512)
                    ps = psp.tile([128, 512], F32, tag="ps",
                                  name=f"ps_{b}_{m}_{hb}")
                    for k in range(KT):
                        wt_h = wta if k < 18 else wtb
                        nc.tensor.matmul(ps[:], wt_h[:, k % 18, :],
                                         xt[:, k, hcols],
                                         start=(k == 0), stop=(k == KT - 1))
                    if m < 5:
                        nc.vector.tensor_copy(qt[:, 2 * m, hcols], ps[:64, :])
                        nc.vector.tensor_copy(qt[:, 2 * m + 1, hcols],
                                              ps[64:128, :])
                    elif m == 5:
                        nc.vector.tensor_copy(kt[:, hcols], ps[:64, :])
                        for j in range(4):
                            r2 = hb * 4 + j
                            pv = psp.tile([128, 72], F32, tag="ps",
                                          name=f"pv_{b}_{r2}")
                            for k in range(KT):
                                wt_h = wta if k < 18 else wtb
                                nc.tensor.matmul(
                                    pv[:, :64],
                                    xt[:, k, r2 * 128:(r2 + 1) * 128],
                                    wt_h[:, k % 18, 64:128],
                                    start=(k == 0), stop=(k == KT - 1))
                            nc.vector.tensor_copy(vt[:, r2, :64], pv[:, :64])
                    else:
                        nc.scalar.activation(gt[:, m - 6, hcols], ps[:],
                                             AF.Gelu)

            # ---- ROPE on qT (10 head tiles) and kT; interleaved into C ----
            def rope(mq):
                tgt = qt[:, mq, :] if mq < NHC else kt[:]
                rot = rotp.tile([64, S], BF16, tag="rot")
                nc.vector.tensor_scalar_mul(rot[0:32, :], tgt[32:64, :], -1.0)
                nc.vector.tensor_copy(rot[32:64, :], tgt[0:32, :])
                nc.vector.tensor_mul(tgt, tgt, cos_sb)
                nc.vector.tensor_mul(rot[:], rot[:], sin_sb)
                nc.vector.tensor_add(tgt, tgt, rot[:])

            rope(NHC)      # kT
            rope(0)

            # ---- Phase C: attention, pipelined over (head, sq-half) units --
            ets = {}
            c2s = {}

            def scores_exp(h, sqc):
                et = et_p.tile([128, 8, 512], BF16, tag="et",
                               name=f"et_{b}_{h}_{sqc}")
                ets[(h, sqc)] = et
                for skt in range(4 * (sqc + 1)):
                    # diag blocks: only sq >= sk columns are ever read
                    lc = skt * 128 - sqc * 512 if skt // 4 == sqc else 0
                    sp = psp.tile([128, 512], F32, tag="ps",
                                  name=f"sp_{b}_{h}_{skt}_{sqc}")
                    nc.tensor.matmul(
                        sp[:, lc:], kt[:, skt * 128:(skt + 1) * 128],
                        qt[:, h, sqc * 512 + lc:(sqc + 1) * 512],
                        start=True, stop=True)
                    if skt // 4 == sqc:
                        nc.vector.tensor_tensor(
                            sp[:, lc:lc + 128], sp[:, lc:lc + 128],
                            dmaskT[:], op=ADD)
                    nc.scalar.activation(et[:, skt, lc:], sp[:, lc:], AF.Exp)

            def ctx_unit(h, sqc):
                et = ets.pop((h, sqc))
                if h % 2 == 0 and sqc == 0:
                    c2s[h // 2] = c2_p.tile([128, 8, 128], BF16, tag="c2",
                                            name=f"c2_{b}_{h}")
                c2 = c2s[h // 2]
                for sqt in range(sqc * 4, sqc * 4 + 4):
                    col = (sqt % 4) * 128
                    cp = psp.tile([128, 72], F32, tag="ps",
                                  name=f"cp_{b}_{h}_{sqt}")
                    for skt in range(sqt + 1):
                        nc.tensor.matmul(
                            cp[:, :65], et[:, skt, col:col + 128],
                            vt[:, skt, :65],
                            start=(skt == 0), stop=(skt == sqt))
                    recd = small.tile([128, 1], F32, tag="recd")
                    nc.vector.reciprocal(recd[:], cp[:, 64:65])
                    nc.vector.tensor_scalar_mul(
                        c2[:, sqt, (h % 2) * 64:(h % 2) * 64 + 64],
                        cp[:, :64], recd[:])
                if h % 2 == 1 and sqc == 1:
                    c2 = c2s.pop(h // 2)
                    nc.scalar.dma_start(
                        ct[:, h // 2, :].rearrange("p (k j) -> p k j", j=128),
                        c2[:], transpose=True)

            units = [(h, sqc) for h in range(NHC) for sqc in range(2)]
            for i, u in enumerate(units):
                scores_exp(*u)
                if u[1] == 0 and u[0] + 1 < NHC:
                    rope(u[0] + 1)   # rope head h+1 behind scores of head h
                if i >= 1:
                    ctx_unit(*units[i - 1])
            ctx_unit(*units[-1])

            # ---- Phase D: dense + down, gelu tiles first, ctx tiles last ----
            for fc in range(FC):
                fcols = slice(fc * 512, (fc + 1) * 512)
                pss = [psp.tile([128, 512], F32, tag="ps",
                                name=f"pd_{b}_{fc}_{i}") for i in range(8)]
                for (k0, k1) in KCH:
                    wdt = wdp.tile([128, k1 - k0, 512], BF16, tag="wdt")
                    nc.gpsimd.dma_start(
                        wdt[:], wdd_d.ap()[fc, k0:k1].rearrange(
                            "k p c -> p k c"))
                    for kk in range(k0, k1):
                        lh = (gt[:, kk] if kk < 18 else ct[:, kk - 18])
                        for r in range(8):
                            tcols = slice(r * 128, (r + 1) * 128)
                            nc.tensor.matmul(pss[r][:], lh[:, tcols],
                                             wdt[:, kk - k0, :],
                                             start=(kk == 0),
                                             stop=(kk == DK - 1))
                for g in range(2):
                    osb = outp.tile([128, 4, 512], BF16, tag="osb")
                    for j in range(4):
                        r = g * 4 + j
                        if g == 0:
                            nc.vector.tensor_copy(osb[:, j, :], pss[r][:])
                        else:
                            nc.scalar.activation(osb[:, j, :], pss[r][:],
                                                 AF.Copy)
                    # store via gpsimd so next batch's x-loads on the SP
                    # queue are not stuck behind these in-order
                    nc.gpsimd.dma_start(
                        out[b * S + g * 512:b * S + (g + 1) * 512,
                            fcols].rearrange("(r p) c -> p r c", p=128),
                        osb[:])
    nc.compile()
    return nc


def _prep_inputs(hidden_states, cos, sin, ln_w1, ln_b1, ln_w2, ln_b2,
                 wq, wk, wv, w_dense, w_h4h, w_4hh):
    f32 = np.float32
    bf = ml_dtypes.bfloat16
    lnw = np.concatenate([np.asarray(ln_w1), np.asarray(ln_w2)]).astype(np.float64)
    lnb = np.concatenate([np.asarray(ln_b1), np.asarray(ln_b2)]).astype(np.float64)

    def pack(Wc, scale=1.0):
        # Wc [O, H] -> [HP, O] f32: ln-folded + bias row + colsum row + zero pad
        W64 = Wc.astype(np.float64) * scale
        Wp = W64 * lnw                      # [O, H]
        bias = W64 @ lnb                    # [O]
        cw = Wp.sum(axis=1)                 # [O]
        O = Wc.shape[0]
        outw = np.zeros((HP, O), f32)
        outw[:H] = Wp.T.astype(f32)
        outw[H] = bias.astype(f32)
        outw[H + 1] = cw.astype(f32)
        return outw

    X = np.asarray(hidden_states, f32).reshape(T, H)
    xb = np.zeros((T, HP), bf)
    xb[:, :H] = X.astype(bf)

    cos2 = np.asarray(cos, f32)[0, 0]       # [S, 64]
    sin2 = np.asarray(sin, f32)[0, 0]
    csn = np.zeros((2, 64, S), bf)
    csn[0] = cos2.T.astype(bf)
    csn[1] = sin2.T.astype(bf)

    # transposed causal mask for scoresT[sk, sq]: keep sk <= sq
    dmask = np.where(np.arange(128)[:, None] <= np.arange(128)[None, :],
                     0.0, NEG).astype(bf)

    wq_pad = np.zeros((NHP * HD, H), f32)
    wq_pad[:NH * HD] = np.asarray(wq, f32)
    wdT_pad = np.zeros((NHP * HD, H), f32)
    wdT_pad[:NH * HD] = np.asarray(w_dense, f32).T
    w14 = np.asarray(w_h4h, f32)
    w41T = np.asarray(w_4hh, f32).T         # [F4, H]

    in_maps = []
    for c in range(8):
        hs = slice(c * QC, (c + 1) * QC)
        fs = slice(c * F4C_REAL, (c + 1) * F4C_REAL)
        wpk = np.zeros((HP, OC), f32)
        wpk[:, :QC] = pack(wq_pad[hs], scale=0.125)
        wpk[:, QC:QC + 64] = pack(np.asarray(wk, f32))
        wpk[:, QC + 64:QC + 128] = pack(np.asarray(wv, f32))
        wpk[:, QC + 128:QC + 128 + F4C_REAL] = pack(w14[fs])
        # retile: [HP, OC] -> [MT, 128, KT*128] (m, p, ko, c) contiguous
        wpk2 = (wpk.reshape(KT, 128, MT, 128).transpose(2, 1, 0, 3)
                .reshape(MT, 128, KT * 128))

        wdd = np.zeros((QC + F4C, HP), f32)
        wdd[:QC, :H] = wdT_pad[hs]
        wdd[QC:QC + F4C_REAL, :H] = w41T[fs]
        # contraction-tile reorder: gelu tiles (5..22) first, ctx (0..4) last
        wdd_r = wdd.reshape(DK, 128, HP)
        perm = list(range(5, 23)) + list(range(5))
        wdd2 = (wdd_r[perm].reshape(DK, 128, FC, 512).transpose(2, 0, 1, 3)
                .copy())
        in_maps.append({
            "xb": xb, "wpk": wpk2.astype(bf), "wdd": wdd2.astype(bf),
            "csn": csn, "dmask": dmask,
        })
    return in_maps


def kernel(hidden_states, attention_mask, cos, sin,
           ln_w1, ln_b1, ln_w2, ln_b2,
           wq, wk, wv, w_dense, w_h4h, w_4hh):
    if "nc" not in _CACHE:
        _CACHE["nc"] = _build()
    nc = _CACHE["nc"]
    in_maps = _prep_inputs(hidden_states, cos, sin, ln_w1, ln_b1, ln_w2, ln_b2,
                           wq, wk, wv, w_dense, w_h4h, w_4hh)
    res = run_bass_kernel_spmd(nc, in_maps, core_ids=list(range(8)))
    acc = np.zeros((T, H), np.float64)
    for r in res.results:
        acc += r["out"][:, :H].astype(np.float64)
    outv = (acc.astype(np.float32)
            + np.asarray(hidden_states, np.float32).reshape(T, H))
    return outv.reshape(B, S, H).astype(np.float32)


# revision 49
# speedup vs baseline: 1.0657x; 1.0657x over previous
"""TRN2 Bass kernel for nn_DecoderLayer_70781061038465 (Falcon-7B style decoder
layer: fractured LayerNorm -> parallel MQA attention + MLP -> residual).

Sharding: 8-way tensor parallelism, no collectives. Each core computes a
partial sum of (attn_out + mlp_out) over its head/MLP shard; the host reduces
the 8 partials and adds the residual.

Per-core math (all LN work folded into matmuls):
  - LN affine folded into projection weights (columns scaled by ln_w; ln_b
    enters via a bias row consumed by an all-ones contraction row).
  - mean/rstd correction folded via (a) pre-scaling token rows by rstd and
    (b) a -mu*rstd contraction row whose weight-row is the column-sum of the
    ln_w-scaled weights.
  - softmax 1/sqrt(64) folded into wq.

Attention runs fully transposed: scoresT[sk,sq] come straight off the PE,
exp is applied without max-subtraction (score range is bounded for this
problem), the softmax denominator rides along as an all-ones 65th column of
V, and normalization happens on the token-major context eviction.

Structure tuned for the TimelineSim cost model:
  - x-tilde and context transposes are single batched XBAR instructions.
  - weights live in DRAM pre-tiled so every weight DMA is one descriptor
    run per partition (contiguous >= 1KB).
  - mlp/dense weight stream issues from the (otherwise idle) GPSIMD queue.
  - attention head loop is software-pipelined (scores/exp of head h issue
    before ctx of head h-1) so the PE never waits on the Act-engine exp.
  - dense contraction runs gelu tiles first, ctx tiles last.
  - output stores are bf16, one DMA per 512-column stripe.
"""
import sys
if "/opt/trn_rl_repo" not in sys.path:
    sys.path.insert(0, "/opt/trn_rl_repo")

from contextlib import ExitStack

import numpy as np
import ml_dtypes

import concourse.bass as bass
import concourse.tile as tile
from concourse import bacc, mybir
from concourse.bass_utils import run_bass_kernel_spmd

F32 = mybir.dt.float32
BF16 = mybir.dt.bfloat16
AX = mybir.AxisListType.X
AF = mybir.ActivationFunctionType
MUL = mybir.AluOpType.mult
ADD = mybir.AluOpType.add

# problem shapes (hardcoded per contract)
B, S, H, NH, HD = 2, 1024, 4544, 71, 64
T = B * S                 # 2048 tokens
HP = 4608                 # padded hidden (36*128)
KT = HP // 128            # 36 contraction tiles
NHP = 80                  # padded heads total
NHC = 10                  # heads per core
QC = NHC * HD             # 640 q channels/core
F4 = 4 * H                # 18176
F4C_REAL = F4 // 8        # 2272
F4C = 2304                # padded (18*128)
OC = QC + 128 + F4C       # 3072 proj out channels (q | k,v | h4h)
MT = OC // 128            # 24 proj m-tiles
DK = (QC + F4C) // 128    # 23 dense+down contraction tiles (18 gelu + 5 ctx)
FC = HP // 512            # 9 output f-chunks
KCH = [(k, min(k + 2, 23)) for k in range(0, 23, 2)]
EPS = 1e-5
NEG = -30.0

_CACHE = {}


def _et_idx(skt, sqc):
    # compact causal score-tile layout: sqc=0 holds skt 0..3, sqc=1 all 8
    return skt if sqc == 0 else 4 + skt


def _build():
    nc = bacc.Bacc("TRN2", target_bir_lowering=False, debug=False)
    xb_d = nc.dram_tensor("xb", [T, HP], BF16, kind="ExternalInput")
    wpk_d = nc.dram_tensor("wpk", [MT, 128, KT * 128], BF16,
                           kind="ExternalInput")
    wdd_d = nc.dram_tensor("wdd", [FC, DK, 128, 512], BF16,
                           kind="ExternalInput")
    cs_d = nc.dram_tensor("csn", [2, 64, S], BF16, kind="ExternalInput")
    dm_d = nc.dram_tensor("dmask", [128, 128], BF16, kind="ExternalInput")
    out_d = nc.dram_tensor("out", [T, HP], BF16, kind="ExternalOutput")

    xb = xb_d.ap()
    out = out_d.ap()

    with tile.TileContext(nc) as tc, ExitStack() as ctx:
        def pool(name, bufs, space="SBUF"):
            return ctx.enter_context(tc.tile_pool(name=name, bufs=bufs, space=space))

        const = pool("const", 1)
        xin = pool("xin", 3)
        xtp = pool("xtp", 1)
        wpool = pool("wp", 3)
        res = pool("res", 1)      # per-batch residents: qt/kt/vt/gt/ct
        et_p = pool("et", 2)
        c2_p = pool("c2", 1)
        rotp = pool("rot", 1)
        wdp = pool("wdp", 2)
        outp = pool("outp", 2)
        small = pool("small", 4)
        psp = pool("psp", 8, space="PSUM")

        cos_sb_t = const.tile([64, S], BF16, tag="cos")
        nc.sync.dma_start(cos_sb_t[:], cs_d.ap()[0])
        sin_sb_t = const.tile([64, S], BF16, tag="sin")
        nc.sync.dma_start(sin_sb_t[:], cs_d.ap()[1])
        cos_sb = cos_sb_t[:]
        sin_sb = sin_sb_t[:]
        dmaskT = const.tile([128, 128], BF16, tag="dmaskT")
        nc.sync.dma_start(dmaskT[:], dm_d.ap())

        for b in range(B):
            qt = res.tile([64, NHC, S], BF16, tag="qt")
            kt = res.tile([64, S], BF16, tag="kt")
            vt = res.tile([128, 8, 72], BF16, tag="vt")
            gt = res.tile([128, 18, S], BF16, tag="gt")
            ct = res.tile([128, 5, S], BF16, tag="ct")
            nc.vector.memset(vt[:, :, 64:65], 1.0)   # denominator ones-column

            # ---- Phase A: LN stats + rstd-scale, half-row pipeline, one
            # batched transpose per half-row ----
            HH = HP // 2              # 2304 = 18 k-tiles
            HB2 = H - HH              # 2240 real cols in second half
            xt = xtp.tile([128, KT, S], BF16, tag="xt")
            for r in range(8):
                row0 = b * S + r * 128
                xra = xin.tile([128, HH], BF16, tag="xrow")
                nc.sync.dma_start(xra[:], xb[row0:row0 + 128, :HH])
                xrb = xin.tile([128, HH], BF16, tag="xrow")
                nc.sync.dma_start(xrb[:], xb[row0:row0 + 128, HH:])
                st = small.tile([128, 16, 6], F32, tag="st")
                xga = xra.rearrange("p (g d) -> p g d", g=8)
                xgb = xrb[:, :HB2].rearrange("p (g d) -> p g d", g=8)
                for g in range(8):
                    nc.vector.bn_stats(st[:, g, :], xga[:, g, :])
                    nc.vector.bn_stats(st[:, 8 + g, :], xgb[:, g, :])
                mv = small.tile([128, 2], F32, tag="mv")
                nc.vector.bn_aggr(mv[:], st[:])
                rstd = small.tile([128, 1], F32, tag="rstd")
                nc.vector.tensor_scalar_add(rstd[:], mv[:, 1:2], EPS)
                nc.scalar.activation(rstd[:], rstd[:], AF.Sqrt)
                nc.vector.reciprocal(rstd[:], rstd[:])
                mr = small.tile([128, 1], F32, tag="mr")
                nc.vector.tensor_tensor(mr[:], mv[:, 0:1], rstd[:], op=MUL)
                nc.vector.tensor_scalar_mul(mr[:], mr[:], -1.0)
                nc.vector.tensor_scalar_mul(xra[:], xra[:], rstd[:])
                nc.scalar.dma_start(xt[:, :18, r * 128:(r + 1) * 128], xra[:],
                                    transpose=True)
                nc.vector.tensor_scalar_mul(xrb[:, :HB2], xrb[:, :HB2],
                                            rstd[:])
                nc.vector.memset(xrb[:, HB2:HB2 + 1], 1.0)
                nc.vector.tensor_copy(xrb[:, HB2 + 1:HB2 + 2], mr[:])
                nc.scalar.dma_start(xt[:, 18:, r * 128:(r + 1) * 128], xrb[:],
                                    transpose=True)

            # ---- Phase B: projections (feature-major q/k/g, token-major v) --
            for m in range(MT):
                wta = wpool.tile([128, 18, 128], BF16, tag="wt")
                nc.sync.dma_start(
                    wta[:], wpk_d.ap()[m][:, :18 * 128].rearrange(
                        "p (k c) -> p k c", c=128))
                wtb = wpool.tile([128, 18, 128], BF16, tag="wt")
                nc.sync.dma_start(
                    wtb[:], wpk_d.ap()[m][:, 18 * 128:].rearrange(
                        "p (k c) -> p k c", c=128))

                for hb in range(2):
                    hcols = slice(hb * 512, hb * 512 + 512)
                    ps = psp.tile([128, 512], F32, tag="ps",
                                  name=f"ps_{b}_{m}_{hb}")
                    for k in range(KT):
                        wt_h = wta if k < 18 else wtb
                        nc.tensor.matmul(ps[:], wt_h[:, k % 18, :],
                                         xt[:, k, hcols],
                                         start=(k == 0), stop=(k == KT - 1))
                    if m < 5:
                        nc.vector.tensor_copy(qt[:, 2 * m, hcols], ps[:64, :])
                        nc.vector.tensor_copy(qt[:, 2 * m + 1, hcols],
                                              ps[64:128, :])
                    elif m == 5:
                        nc.vector.tensor_copy(kt[:, hcols], ps[:64, :])
                        for j in range(4):
                            r2 = hb * 4 + j
                            pv = psp.tile([128, 72], F32, tag="ps",
                                          name=f"pv_{b}_{r2}")
                            for k in range(KT):
                                wt_h = wta if k < 18 else wtb
                                nc.tensor.matmul(
                                    pv[:, :64],
                                    xt[:, k, r2 * 128:(r2 + 1) * 128],
                                    wt_h[:, k % 18, 64:128],
                                    start=(k == 0), stop=(k == KT - 1))
                            nc.vector.tensor_copy(vt[:, r2, :64], pv[:, :64])
                    else:
                        nc.scalar.activation(gt[:, m - 6, hcols], ps[:],
                                             AF.Gelu)

            # ---- ROPE on qT (10 head tiles) and kT; interleaved into C ----
            def rope(mq):
                tgt = qt[:, mq, :] if mq < NHC else kt[:]
                rot = rotp.tile([64, S], BF16, tag="rot")
                nc.vector.tensor_scalar_mul(rot[0:32, :], tgt[32:64, :], -1.0)
                nc.vector.tensor_copy(rot[32:64, :], tgt[0:32, :])
                nc.vector.tensor_mul(tgt, tgt, cos_sb)
                nc.vector.tensor_mul(rot[:], rot[:], sin_sb)
                nc.vector.tensor_add(tgt, tgt, rot[:])

            rope(NHC)      # kT
            rope(0)

            # ---- Phase C: attention, pipelined over (head, sq-half) units --
            ets = {}
            c2s = {}

            def scores_exp(h, sqc):
                et = et_p.tile([128, 8, 512], BF16, tag="et",
                               name=f"et_{b}_{h}_{sqc}")
                ets[(h, sqc)] = et
                for skt in range(4 * (sqc + 1)):
                    # diag blocks: only sq >= sk columns are ever read
                    lc = skt * 128 - sqc * 512 if skt // 4 == sqc else 0
                    sp = psp.tile([128, 512], F32, tag="ps",
                                  name=f"sp_{b}_{h}_{skt}_{sqc}")
                    nc.tensor.matmul(
                        sp[:, lc:], kt[:, skt * 128:(skt + 1) * 128],
                        qt[:, h, sqc * 512 + lc:(sqc + 1) * 512],
                        start=True, stop=True)
                    if skt // 4 == sqc:
                        nc.vector.tensor_tensor(
                            sp[:, lc:lc + 128], sp[:, lc:lc + 128],
                            dmaskT[:], op=ADD)
                    nc.scalar.activation(et[:, skt, lc:], sp[:, lc:], AF.Exp)

            def ctx_unit(h, sqc):
                et = ets.pop((h, sqc))
                if h % 2 == 0 and sqc == 0:
                    c2s[h // 2] = c2_p.tile([128, 8, 128], BF16, tag="c2",
                                            name=f"c2_{b}_{h}")
                c2 = c2s[h // 2]
                for sqt in range(sqc * 4, sqc * 4 + 4):
                    col = (sqt % 4) * 128
                    cp = psp.tile([128, 72], F32, tag="ps",
                                  name=f"cp_{b}_{h}_{sqt}")
                    for skt in range(sqt + 1):
                        nc.tensor.matmul(
                            cp[:, :65], et[:, skt, col:col + 128],
                            vt[:, skt, :65],
                            start=(skt == 0), stop=(skt == sqt))
                    recd = small.tile([128, 1], F32, tag="recd")
                    nc.vector.reciprocal(recd[:], cp[:, 64:65])
                    nc.vector.tensor_scalar_mul(
                        c2[:, sqt, (h % 2) * 64:(h % 2) * 64 + 64],
                        cp[:, :64], recd[:])
                if h % 2 == 1 and sqc == 1:
                    c2 = c2s.pop(h // 2)
                    nc.scalar.dma_start(
                        ct[:, h // 2, :].rearrange("p (k j) -> p k j", j=128),
                        c2[:], transpose=True)

            units = [(h, sqc) for h in range(NHC) for sqc in range(2)]
            for i, u in enumerate(units):
                scores_exp(*u)
                if u[1] == 0 and u[0] + 1 < NHC:
                    rope(u[0] + 1)   # rope head h+1 behind scores of head h
                if i >= 1:
                    ctx_unit(*units[i - 1])
            ctx_unit(*units[-1])

            # ---- Phase D: dense + down, gelu tiles first, ctx tiles last ----
            for fc in range(FC):
                fcols = slice(fc * 512, (fc + 1) * 512)
                pss = [psp.tile([128, 512], F32, tag="ps",
                                name=f"pd_{b}_{fc}_{i}") for i in range(8)]
                for (k0, k1) in KCH:
                    wdt = wdp.tile([128, k1 - k0, 512], BF16, tag="wdt")
                    nc.gpsimd.dma_start(
                        wdt[:], wdd_d.ap()[fc, k0:k1].rearrange(
                            "k p c -> p k c"))
                    for kk in range(k0, k1):
                        lh = (gt[:, kk] if kk < 18 else ct[:, kk - 18])
                        for r in range(8):
                            tcols = slice(r * 128, (r + 1) * 128)
                            nc.tensor.matmul(pss[r][:], lh[:, tcols],
                                             wdt[:, kk - k0, :],
                                             start=(kk == 0),
                                             stop=(kk == DK - 1))
                for g in range(2):
                    osb = outp.tile([128, 4, 512], BF16, tag="osb")
                    for j in range(4):
                        r = g * 4 + j
                        if g == 0:
                            nc.vector.tensor_copy(osb[:, j, :], pss[r][:])
                        else:
                            nc.scalar.activation(osb[:, j, :], pss[r][:],
                                                 AF.Copy)
                    nc.sync.dma_start(
                        out[b * S + g * 512:b * S + (g + 1) * 512,
                            fcols].rearrange("(r p) c -> p r c", p=128),
                        osb[:])
    nc.compile()
    return nc


def _prep_inputs(hidden_states, cos, sin, ln_w1, ln_b1, ln_w2, ln_b2,
                 wq, wk, wv, w_dense, w_h4h, w_4hh):
    f32 = np.float32
    bf = ml_dtypes.bfloat16
    lnw = np.concatenate([np.asarray(ln_w1), np.asarray(ln_w2)]).astype(np.float64)
    lnb = np.concatenate([np.asarray(ln_b1), np.asarray(ln_b2)]).astype(np.float64)

    def pack(Wc, scale=1.0):
        # Wc [O, H] -> [HP, O] f32: ln-folded + bias row + colsum row + zero pad
        W64 = Wc.astype(np.float64) * scale
        Wp = W64 * lnw                      # [O, H]
        bias = W64 @ lnb                    # [O]
        cw = Wp.sum(axis=1)                 # [O]
        O = Wc.shape[0]
        outw = np.zeros((HP, O), f32)
        outw[:H] = Wp.T.astype(f32)
        outw[H] = bias.astype(f32)
        outw[H + 1] = cw.astype(f32)
        return outw

    X = np.asarray(hidden_states, f32).reshape(T, H)
    xb = np.zeros((T, HP), bf)
    xb[:, :H] = X.astype(bf)

    cos2 = np.asarray(cos, f32)[0, 0]       # [S, 64]
    sin2 = np.asarray(sin, f32)[0, 0]
    csn = np.zeros((2, 64, S), bf)
    csn[0] = cos2.T.astype(bf)
    csn[1] = sin2.T.astype(bf)

    # transposed causal mask for scoresT[sk, sq]: keep sk <= sq
    dmask = np.where(np.arange(128)[:, None] <= np.arange(128)[None, :],
                     0.0, NEG).astype(bf)

    wq_pad = np.zeros((NHP * HD, H), f32)
    wq_pad[:NH * HD] = np.asarray(wq, f32)
    wdT_pad = np.zeros((NHP * HD, H), f32)
    wdT_pad[:NH * HD] = np.asarray(w_dense, f32).T
    w14 = np.asarray(w_h4h, f32)
    w41T = np.asarray(w_4hh, f32).T         # [F4, H]

    in_maps = []
    for c in range(8):
        hs = slice(c * QC, (c + 1) * QC)
        fs = slice(c * F4C_REAL, (c + 1) * F4C_REAL)
        wpk = np.zeros((HP, OC), f32)
        wpk[:, :QC] = pack(wq_pad[hs], scale=0.125)
        wpk[:, QC:QC + 64] = pack(np.asarray(wk, f32))
        wpk[:, QC + 64:QC + 128] = pack(np.asarray(wv, f32))
        wpk[:, QC + 128:QC + 128 + F4C_REAL] = pack(w14[fs])
        # retile: [HP, OC] -> [MT, 128, KT*128] (m, p, ko, c) contiguous
        wpk2 = (wpk.reshape(KT, 128, MT, 128).transpose(2, 1, 0, 3)
                .reshape(MT, 128, KT * 128))

        wdd = np.zeros((QC + F4C, HP), f32)
        wdd[:QC, :H] = wdT_pad[hs]
        wdd[QC:QC + F4C_REAL, :H] = w41T[fs]
        # contraction-tile reorder: gelu tiles (5..22) first, ctx (0..4) last
        wdd_r = wdd.reshape(DK, 128, HP)
        perm = list(range(5, 23)) + list(range(5))
        wdd2 = (wdd_r[perm].reshape(DK, 128, FC, 512).transpose(2, 0, 1, 3)
                .copy())
        in_maps.append({
            "xb": xb, "wpk": wpk2.astype(bf), "wdd": wdd2.astype(bf),
            "csn": csn, "dmask": dmask,
        })
    return in_maps


def kernel(hidden_states, attention_mask, cos, sin,
           ln_w1, ln_b1, ln_w2, ln_b2,
           wq, wk, wv, w_dense, w_h4h, w_4hh):
    if "nc" not in _CACHE:
        _CACHE["nc"] = _build()
    nc = _CACHE["nc"]
    in_maps = _prep_inputs(hidden_states, cos, sin, ln_w1, ln_b1, ln_w2, ln_b2,
                           wq, wk, wv, w_dense, w_h4h, w_4hh)
    res = run_bass_kernel_spmd(nc, in_maps, core_ids=list(range(8)))
    acc = np.zeros((T, H), np.float64)
    for r in res.results:
        acc += r["out"][:, :H].astype(np.float64)
    outv = (acc.astype(np.float32)
            + np.asarray(hidden_states, np.float32).reshape(T, H))
    return outv.reshape(B, S, H).astype(np.float32)
